# revision 10
# baseline (speedup 1.0000x reference)
"""GCN VGAE encoder (nn_Encoder_25065429139538) on 8 Trainium2 NeuronCores.

Strategy (sharding_hint: shard nodes across cores, partition edges by dst,
replicate weights):
  - Nodes padded to 100352 = 8 x 12544; core d owns dst rows [d*SH, (d+1)*SH).
  - Per-edge normalization coefficients (symmetric GCN norm, incl. self-loops)
    are folded into a single per-edge scalar on the host; duplicate (src,dst)
    pairs are merged. Edges are grouped per core by dst 128-row block (for
    PSUM-accumulated segment sums) and by src quarter (so dma_gather's int16
    indices address a <32768-row subtable).
  - Aggregation commutes with the dense projections, so each layer gathers raw
    table rows h[src] (dma_gather, 4 SWDGE queues), scales them by the edge
    coefficient (DVE), and reduces segments with a one-hot matmul on the
    TensorEngine accumulating in PSUM -- no scatter DMA at all.
  - The projection W then runs on the core's 12544 aggregated rows only; h is
    AllGather'd between layers to rebuild the full gather table. mu/logstd
    share one aggregation pass (both use the unweighted norm and h2).
"""

import math

import numpy as np

import concourse.bass as bass
import concourse.bacc as bacc
import concourse.mybir as mybir
import concourse.tile as tile
from concourse.bass_utils import run_bass_kernel_spmd
from concourse.library_config import mlp

# ---- problem constants (hardcoded per contract) ----
N = 100000
FIN, HID, OUT = 128, 64, 32
NCORES = 8

# ---- layout constants ----
SH = 12544            # rows per core (100352 / 8)
NPAD = SH * NCORES    # 100352
NBLK = SH // 128      # 98 dst blocks per core
NSUB = 4              # src subtables (int16 gather indices)
SUB = NPAD // NSUB    # 25088 rows per subtable
SLOTS = 1024          # gather slots per dma_gather instruction
CPG = SLOTS // 128    # chunks per gather group = 8


def _wrap_idx(slots_i16):
    """[G*1024] int16 -> [G, 128, 64]: slot i at [i%16 (+16m replicas), i//16]."""
    g = slots_i16.reshape(-1, SLOTS // 16, 16)          # [G, 64, 16]
    g = np.swapaxes(g, 1, 2)                            # [G, 16, 64]
    return np.tile(g, (1, 8, 1)).astype(np.int16)       # [G, 128, 64]


def _prep(x, edge_index, edge_weight):
    """Host-side edge partitioning. Returns (structure, shared arrays, per-core arrays)."""
    src = np.asarray(edge_index[0], dtype=np.int64)
    dst = np.asarray(edge_index[1], dtype=np.int64)
    ew = np.asarray(edge_weight, dtype=np.float32)

    deg_w = np.zeros(N, np.float32)
    np.add.at(deg_w, dst, ew)
    deg_w += 1.0  # self-loop weight
    deg_1 = (np.bincount(dst, minlength=N) + 1).astype(np.float32)
    dinv_w = 1.0 / np.sqrt(deg_w)
    dinv_1 = 1.0 / np.sqrt(deg_1)

    nw = dinv_w[src] * ew * dinv_w[dst]
    n1 = dinv_1[src] * dinv_1[dst]
    vs = np.arange(N, dtype=np.int64)
    src_a = np.concatenate([src, vs])
    dst_a = np.concatenate([dst, vs])
    nw_a = np.concatenate([nw, dinv_w * dinv_w])
    n1_a = np.concatenate([n1, dinv_1 * dinv_1])

    # merge duplicate (src, dst) pairs
    key = src_a * NPAD + dst_a
    ukey, inv = np.unique(key, return_inverse=True)
    unw = np.zeros(len(ukey), np.float32)
    un1 = np.zeros(len(ukey), np.float32)
    np.add.at(unw, inv, nw_a)
    np.add.at(un1, inv, n1_a)
    usrc = ukey // NPAD
    udst = ukey % NPAD

    core = udst // SH
    t_all = (udst % SH) // 128
    dloc_all = (udst % SH) % 128
    s_all = usrc // SUB
    sloc_all = usrc % SUB

    # per (core, t, s) edge counts -> shared chunk structure K_ts
    cell = (core * NBLK + t_all) * NSUB + s_all
    cnt = np.bincount(cell, minlength=NCORES * NBLK * NSUB).reshape(NCORES, NBLK, NSUB)
    K_ts = np.maximum(1, np.ceil(cnt.max(axis=0) / 128).astype(np.int64))  # [NBLK, NSUB]
    C_s = K_ts.sum(axis=0)                      # chunks per s-stream
    G_s = [int(math.ceil(int(c) / CPG)) for c in C_s]
    base_pos = np.zeros((NBLK, NSUB), np.int64)  # chunk stream position of (t,s)
    for s in range(NSUB):
        base_pos[:, s] = np.cumsum(np.concatenate([[0], K_ts[:-1, s]]))

    per_core = []
    order = np.lexsort((sloc_all, s_all, t_all, core))
    osrc, osloc, os_, ot, odloc, onw, on1, ocore = (
        usrc[order], sloc_all[order], s_all[order], t_all[order],
        dloc_all[order], unw[order], un1[order], core[order],
    )
    cstart = np.searchsorted(ocore, np.arange(NCORES + 1))
    for d in range(NCORES):
        lo, hi = cstart[d], cstart[d + 1]
        dt, ds = ot[lo:hi], os_[lo:hi]
        dsl, ddl = osloc[lo:hi], odloc[lo:hi]
        dnw, dn1 = onw[lo:hi], on1[lo:hi]
        srcw, dlocv, nwv, n1v = [], [], [], []
        # cell start offsets within this core's slice (t-major, s-minor sorted)
        cell_d = dt * NSUB + ds
        cello = np.searchsorted(cell_d, np.arange(NBLK * NSUB + 1))
        for s in range(NSUB):
            nslot = G_s[s] * SLOTS
            sl = np.zeros(nslot, np.int64)
            dl = np.zeros(nslot, np.float32)
            wv = np.zeros(nslot, np.float32)
            v1 = np.zeros(nslot, np.float32)
            for t in range(NBLK):
                a, b = cello[t * NSUB + s], cello[t * NSUB + s + 1]
                n = b - a
                p0 = base_pos[t, s] * 128
                sl[p0:p0 + n] = dsl[a:b]
                dl[p0:p0 + n] = ddl[a:b]
                wv[p0:p0 + n] = dnw[a:b]
                v1[p0:p0 + n] = dn1[a:b]
            srcw.append(_wrap_idx(sl.astype(np.int16)))
            dlocv.append(dl.reshape(G_s[s], CPG, 128).transpose(0, 2, 1).copy())
            nwv.append(wv.reshape(G_s[s], CPG, 128).transpose(0, 2, 1).copy())
            n1v.append(v1.reshape(G_s[s], CPG, 128).transpose(0, 2, 1).copy())
        per_core.append((srcw, dlocv, nwv, n1v))

    return K_ts, G_s, base_pos, per_core


def _build(K_ts, G_s, base_pos):
    f32 = mybir.dt.float32
    nc = bacc.Bacc(None, target_bir_lowering=False, num_swdge_queues=4, num_devices=NCORES)

    x_d = nc.dram_tensor("x", [NPAD, FIN], f32, kind="ExternalInput")
    srcw_d = [nc.dram_tensor(f"srcw{s}", [G_s[s], 128, SLOTS // 16], mybir.dt.int16, kind="ExternalInput") for s in range(NSUB)]
    dloc_d = [nc.dram_tensor(f"dloc{s}", [G_s[s], 128, CPG], f32, kind="ExternalInput") for s in range(NSUB)]
    nw_d = [nc.dram_tensor(f"nw{s}", [G_s[s], 128, CPG], f32, kind="ExternalInput") for s in range(NSUB)]
    n1_d = [nc.dram_tensor(f"n1{s}", [G_s[s], 128, CPG], f32, kind="ExternalInput") for s in range(NSUB)]
    w1_d = nc.dram_tensor("W1", [FIN, HID], f32, kind="ExternalInput")
    w2_d = nc.dram_tensor("W2", [HID, HID], f32, kind="ExternalInput")
    wm_d = nc.dram_tensor("Wmu", [HID, OUT], f32, kind="ExternalInput")
    wl_d = nc.dram_tensor("Wls", [HID, OUT], f32, kind="ExternalInput")
    b1_d = nc.dram_tensor("b1", [HID, 1], f32, kind="ExternalInput")
    b2_d = nc.dram_tensor("b2", [HID, 1], f32, kind="ExternalInput")
    bm_d = nc.dram_tensor("bmu", [OUT, 1], f32, kind="ExternalInput")
    bl_d = nc.dram_tensor("bls", [OUT, 1], f32, kind="ExternalInput")
    iota_d = nc.dram_tensor("iota8", [128, CPG * 128], f32, kind="ExternalInput")
    ident_d = nc.dram_tensor("ident", [128, 128], f32, kind="ExternalInput")
    out_d = nc.dram_tensor("out", [SH, 2 * OUT], f32, kind="ExternalOutput")

    ag_in = [nc.dram_tensor(f"ag_in{i}", [SH, HID], f32) for i in range(2)]
    tables = [nc.dram_tensor(f"table{i}", [NPAD, HID], f32, addr_space="Shared") for i in range(2)]

    with tile.TileContext(nc) as tc:
        with (
            tc.tile_pool(name="const", bufs=1) as kpool,
            tc.tile_pool(name="idx", bufs=8) as ipool,
            tc.tile_pool(name="meta", bufs=8) as mpool,
            tc.tile_pool(name="g", bufs=2) as gpool,
            tc.tile_pool(name="b", bufs=2) as bpool,
            tc.tile_pool(name="agg", bufs=1) as apool,
            tc.tile_pool(name="stage", bufs=1) as spool,
            tc.tile_pool(name="tmp", bufs=4) as tpool,
            tc.tile_pool(name="pagg", bufs=2, space="PSUM") as pagg,
            tc.tile_pool(name="pmm", bufs=2, space="PSUM") as pmm,
            tc.tile_pool(name="ptr", bufs=2, space="PSUM") as ptr,
        ):
            nc.gpsimd.load_library(mlp)

            iota_t = kpool.tile([128, CPG * 128], f32)
            nc.sync.dma_start(iota_t[:], iota_d[:])
            ident_t = kpool.tile([128, 128], f32)
            nc.sync.dma_start(ident_t[:], ident_d[:])
            w1_t = kpool.tile([FIN, HID], f32)
            nc.sync.dma_start(w1_t[:], w1_d[:])
            w2_t = kpool.tile([HID, HID], f32)
            nc.sync.dma_start(w2_t[:], w2_d[:])
            wm_t = kpool.tile([HID, OUT], f32)
            nc.sync.dma_start(wm_t[:], wm_d[:])
            wl_t = kpool.tile([HID, OUT], f32)
            nc.sync.dma_start(wl_t[:], wl_d[:])
            b1_t = kpool.tile([HID, 1], f32)
            nc.sync.dma_start(b1_t[:], b1_d[:])
            b2_t = kpool.tile([HID, 1], f32)
            nc.sync.dma_start(b2_t[:], b2_d[:])
            bm_t = kpool.tile([OUT, 1], f32)
            nc.sync.dma_start(bm_t[:], bm_d[:])
            bl_t = kpool.tile([OUT, 1], f32)
            nc.sync.dma_start(bl_t[:], bl_d[:])

            def aggregate(F, table_of_s, norm_of_s, agg_t):
                """agg_t[:F, t*128+v] = sum over edges into (t,v) of coeff*table[src]."""
                cur = {}  # s -> (g, gtile, btile)
                gq = [0]  # emission counter: keeps Tile's DMASW lane (i%8) and
                          # our SWDGE queue (i%4) consistent so sems stay
                          # queue-locked correctly

                def ensure(s, g):
                    if s in cur and cur[s][0] == g:
                        return cur[s][1]
                    it = ipool.tile([128, SLOTS // 16], mybir.dt.int16, tag="idx")
                    nc.sync.dma_start(it[:], srcw_d[s][g])
                    gt = gpool.tile([128, CPG, F], f32, tag=f"g{s}")
                    nc.gpsimd.dma_gather(
                        gt[:], table_of_s(s), it[:], SLOTS, SLOTS, F,
                        queue_num=gq[0] % 4,
                    )
                    gq[0] += 1
                    nt = mpool.tile([128, CPG], f32, tag="nm")
                    nc.sync.dma_start(nt[:], norm_of_s(s)[g])
                    dt_ = mpool.tile([128, CPG], f32, tag="dl")
                    nc.sync.dma_start(dt_[:], dloc_d[s][g])
                    bt = bpool.tile([128, CPG, 128], f32, tag=f"b{s}")
                    nc.vector.tensor_tensor(
                        out=bt[:],
                        in0=iota_t[:].rearrange("p (j v) -> p j v", j=CPG),
                        in1=dt_[:].to_broadcast([128, CPG, 128]),
                        op=mybir.AluOpType.is_equal,
                    )
                    nc.vector.tensor_tensor(
                        out=gt[:],
                        in0=gt[:],
                        in1=nt[:].to_broadcast([128, CPG, F]),
                        op=mybir.AluOpType.mult,
                    )
                    cur[s] = (g, (gt, bt))
                    return gt, bt

                for t in range(NBLK):
                    ps = pagg.tile([128, 128], f32, tag="pagg")
                    nchunks = int(K_ts[t].sum())
                    ci = 0
                    for s in range(NSUB):
                        for k in range(int(K_ts[t, s])):
                            pos = int(base_pos[t, s]) + k
                            g, j = divmod(pos, CPG)
                            gt, bt = ensure(s, g)
                            nc.tensor.matmul(
                                ps[:F, :],
                                lhsT=gt[:, j, :],
                                rhs=bt[:, j, :],
                                start=(ci == 0),
                                stop=(ci == nchunks - 1),
                            )
                            ci += 1
                    nc.vector.tensor_copy(agg_t[:F, t * 128:(t + 1) * 128], ps[:F, :])

            def project_transpose_store(agg_t, F, w_t, b_t, func, dram_rows):
                """dram[t*128+p, :] = func(W.T @ agg[:, t*128+p] + b), per block."""
                st = spool.tile([128, NBLK * HID], f32, tag="st")
                for t in range(NBLK):
                    ph = pmm.tile([HID, 128], f32, tag="pmm")
                    nc.tensor.matmul(
                        ph[:], lhsT=w_t[:], rhs=agg_t[:F, t * 128:(t + 1) * 128],
                        start=True, stop=True,
                    )
                    ht = tpool.tile([HID, 128], f32, tag="ht")
                    if func is None:
                        nc.vector.tensor_scalar_add(ht[:], ph[:], b_t[:])
                    else:
                        nc.scalar.activation(ht[:], ph[:], func, bias=b_t[:])
                    pt = ptr.tile([128, HID], f32, tag="ptr")
                    nc.tensor.transpose(pt[:], ht[:], ident_t[:HID, :HID])
                    nc.vector.tensor_copy(st[:, t * HID:(t + 1) * HID], pt[:])
                nc.sync.dma_start(
                    dram_rows.rearrange("(t p) f -> p t f", p=128),
                    st[:].rearrange("p (t f) -> p t f", f=HID),
                )

            # ---- layer 1: gather raw x ----
            agg1 = apool.tile([128, NBLK * 128], f32, tag="agg")
            aggregate(FIN, lambda s: x_d[s * SUB:(s + 1) * SUB, :], lambda s: nw_d[s], agg1)
            project_transpose_store(agg1, FIN, w1_t, b1_t, mybir.ActivationFunctionType.Relu, ag_in[0][:])
            nc.gpsimd.collective_compute(
                "AllGather", mybir.AluOpType.bypass,
                replica_groups=[list(range(NCORES))],
                ins=[ag_in[0][:]], outs=[tables[0][:]],
            )

            # ---- layer 2 ----
            agg2 = apool.tile([128, NBLK * 128], f32, tag="agg")
            aggregate(HID, lambda s: tables[0][s * SUB:(s + 1) * SUB, :], lambda s: nw_d[s], agg2)
            project_transpose_store(agg2, HID, w2_t, b2_t, None, ag_in[1][:])
            nc.gpsimd.collective_compute(
                "AllGather", mybir.AluOpType.bypass,
                replica_groups=[list(range(NCORES))],
                ins=[ag_in[1][:]], outs=[tables[1][:]],
            )

            # ---- layer 3: mu / logstd (shared aggregation) ----
            agg3 = apool.tile([128, NBLK * 128], f32, tag="agg")
            aggregate(HID, lambda s: tables[1][s * SUB:(s + 1) * SUB, :], lambda s: n1_d[s], agg3)
            st3 = spool.tile([128, NBLK * 2 * OUT], f32, tag="st")
            for t in range(NBLK):
                mlt = tpool.tile([2 * OUT, 128], f32, tag="ht")
                pm = pmm.tile([OUT, 128], f32, tag="pmm")
                nc.tensor.matmul(pm[:], lhsT=wm_t[:], rhs=agg3[:HID, t * 128:(t + 1) * 128], start=True, stop=True)
                nc.vector.tensor_scalar_add(mlt[:OUT, :], pm[:], bm_t[:])
                pl = pmm.tile([OUT, 128], f32, tag="pmm")
                nc.tensor.matmul(pl[:], lhsT=wl_t[:], rhs=agg3[:HID, t * 128:(t + 1) * 128], start=True, stop=True)
                nc.vector.tensor_scalar_add(mlt[OUT:2 * OUT, :], pl[:], bl_t[:])
                pt = ptr.tile([128, 2 * OUT], f32, tag="ptr")
                nc.tensor.transpose(pt[:], mlt[:], ident_t[:2 * OUT, :2 * OUT])
                nc.vector.tensor_copy(st3[:, t * 2 * OUT:(t + 1) * 2 * OUT], pt[:])
            nc.sync.dma_start(
                out_d[:].rearrange("(t p) f -> p t f", p=128),
                st3[:].rearrange("p (t f) -> p t f", f=2 * OUT),
            )

    # Tile round-robins Pool-DMA completion sems over 8 DMASW lanes without
    # queue awareness, but each sem is hardware-locked to the first SWDGE
    # queue that increments it. Rewrite each gather's queue to lane % 4 so
    # every lane's sem is only ever incremented from one queue.
    for fn in nc.m.functions:
        for blk in fn.blocks:
            for ins in blk.instructions:
                if isinstance(ins, mybir.InstDMAGatherAnt) and ins.sync_info:
                    for u in ins.sync_info.on_update:
                        name = getattr(u, "ant_name", "") or ""
                        if name.startswith("DMASW"):
                            ins.queue_num = int(name[5:].split("_")[0]) % 4
                            break

    nc.compile()
    return nc


def _run(inputs, trace=False):
    x = np.asarray(inputs["x"], np.float32)
    K_ts, G_s, base_pos, per_core = _prep(
        x, np.asarray(inputs["edge_index"]), np.asarray(inputs["edge_weight"])
    )
    nc = _build(K_ts, G_s, base_pos)

    x_pad = np.zeros((NPAD, FIN), np.float32)
    x_pad[:N] = x
    iota8 = np.tile(np.arange(128, dtype=np.float32)[None, :], (128, CPG)).reshape(128, CPG * 128)
    shared = {
        "x": x_pad,
        "W1": np.asarray(inputs["W1"], np.float32),
        "W2": np.asarray(inputs["W2"], np.float32),
        "Wmu": np.asarray(inputs["Wmu"], np.float32),
        "Wls": np.asarray(inputs["Wls"], np.float32),
        "b1": np.asarray(inputs["b1"], np.float32).reshape(HID, 1),
        "b2": np.asarray(inputs["b2"], np.float32).reshape(HID, 1),
        "bmu": np.asarray(inputs["bmu"], np.float32).reshape(OUT, 1),
        "bls": np.asarray(inputs["bls"], np.float32).reshape(OUT, 1),
        "iota8": iota8,
        "ident": np.eye(128, dtype=np.float32),
    }
    in_maps = []
    for d in range(NCORES):
        srcw, dlocv, nwv, n1v = per_core[d]
        m = dict(shared)
        for s in range(NSUB):
            m[f"srcw{s}"] = srcw[s]
            m[f"dloc{s}"] = dlocv[s]
            m[f"nw{s}"] = nwv[s]
            m[f"n1{s}"] = n1v[s]
        in_maps.append(m)

    res = run_bass_kernel_spmd(nc, in_maps, core_ids=list(range(NCORES)), trace=trace)
    full = np.concatenate([res.results[d]["out"] for d in range(NCORES)], axis=0)
    mu = full[:N, :OUT].copy()
    logstd = full[:N, OUT:].copy()
    return (mu, logstd), res


def kernel(**inputs):
    (mu, logstd), _ = _run(inputs, trace=False)
    return mu, logstd


# revision 14
# speedup vs baseline: 1.2846x; 1.2846x over previous
"""GCN VGAE encoder (nn_Encoder_25065429139538) on 8 Trainium2 NeuronCores.

Strategy (sharding_hint: shard nodes across cores, partition edges by dst,
replicate weights):
  - Nodes padded to 100352 = 8 x 12544; core d owns dst rows [d*SH, (d+1)*SH).
  - Per-edge normalization coefficients (symmetric GCN norm, incl. self-loops)
    are folded into a single per-edge scalar on the host; duplicate (src,dst)
    pairs are merged. Edges are grouped per core by dst 128-row block (for
    PSUM-accumulated segment sums) and by src quarter (so dma_gather's int16
    indices address a <32768-row subtable).
  - Aggregation commutes with the dense projections, so each layer gathers raw
    table rows h[src] (dma_gather, 4 SWDGE queues), scales them by the edge
    coefficient (DVE), and reduces segments with a one-hot matmul on the
    TensorEngine accumulating in PSUM -- no scatter DMA at all.
  - The projection W then runs on the core's 12544 aggregated rows only; h is
    AllGather'd between layers to rebuild the full gather table. mu/logstd
    share one aggregation pass (both use the unweighted norm and h2).
"""

import math

import numpy as np

import concourse.bass as bass
import concourse.bacc as bacc
import concourse.mybir as mybir
import concourse.tile as tile
from concourse.bass_utils import run_bass_kernel_spmd
from concourse.library_config import mlp

# ---- problem constants (hardcoded per contract) ----
N = 100000
FIN, HID, OUT = 128, 64, 32
NCORES = 8

# ---- layout constants ----
SH = 12544            # rows per core (100352 / 8)
NPAD = SH * NCORES    # 100352
NBLK = SH // 128      # 98 dst blocks per core
NSUB = 4              # src subtables (int16 gather indices)
SUB = NPAD // NSUB    # 25088 rows per subtable
SLOTS = 1024          # gather slots per dma_gather instruction
CPG = SLOTS // 128    # chunks per gather group = 8


def _wrap_idx(slots_i16):
    """[G*1024] int16 -> [G, 128, 64]: slot i at [i%16 (+16m replicas), i//16]."""
    g = slots_i16.reshape(-1, SLOTS // 16, 16)          # [G, 64, 16]
    g = np.swapaxes(g, 1, 2)                            # [G, 16, 64]
    return np.tile(g, (1, 8, 1)).astype(np.int16)       # [G, 128, 64]


def _prep(x, edge_index, edge_weight):
    """Host-side edge partitioning. Returns (structure, per-core arrays)."""
    src = np.asarray(edge_index[0], dtype=np.int64)
    dst = np.asarray(edge_index[1], dtype=np.int64)
    ew = np.asarray(edge_weight, dtype=np.float32)

    deg_w = np.zeros(N, np.float32)
    np.add.at(deg_w, dst, ew)
    deg_w += 1.0  # self-loop weight
    deg_1 = (np.bincount(dst, minlength=N) + 1).astype(np.float32)
    dinv_w = 1.0 / np.sqrt(deg_w)
    dinv_1 = 1.0 / np.sqrt(deg_1)

    nw = dinv_w[src] * ew * dinv_w[dst]
    n1 = dinv_1[src] * dinv_1[dst]

    # merge duplicate (src, dst) pairs (self-loops are handled separately via
    # a dense per-shard stream, so only real edges here)
    key = src * NPAD + dst
    ukey, inv = np.unique(key, return_inverse=True)
    unw = np.zeros(len(ukey), np.float32)
    un1 = np.zeros(len(ukey), np.float32)
    np.add.at(unw, inv, nw)
    np.add.at(un1, inv, n1)
    usrc = ukey // NPAD
    udst = ukey % NPAD

    core = udst // SH
    t_all = (udst % SH) // 128
    dloc_all = (udst % SH) % 128
    s_all = usrc // SUB
    sloc_all = usrc % SUB

    # per (core, t, s) edge counts -> shared chunk structure K_ts
    cell = (core * NBLK + t_all) * NSUB + s_all
    cnt = np.bincount(cell, minlength=NCORES * NBLK * NSUB).reshape(NCORES, NBLK, NSUB)
    K_ts = np.maximum(1, np.ceil(cnt.max(axis=0) / 128).astype(np.int64))  # [NBLK, NSUB]
    C_s = K_ts.sum(axis=0)                      # chunks per s-stream
    G_s = [int(math.ceil(int(c) / CPG)) for c in C_s]
    base_pos = np.zeros((NBLK, NSUB), np.int64)  # chunk stream position of (t,s)
    for s in range(NSUB):
        base_pos[:, s] = np.cumsum(np.concatenate([[0], K_ts[:-1, s]]))

    G5 = int(math.ceil(NBLK / CPG))  # self-loop meta groups

    per_core = []
    order = np.lexsort((sloc_all, s_all, t_all, core))
    osloc, os_, ot, odloc, onw, on1, ocore = (
        sloc_all[order], s_all[order], t_all[order],
        dloc_all[order], unw[order], un1[order], core[order],
    )
    cstart = np.searchsorted(ocore, np.arange(NCORES + 1))
    for d in range(NCORES):
        lo, hi = cstart[d], cstart[d + 1]
        dt, ds = ot[lo:hi], os_[lo:hi]
        dsl, ddl = osloc[lo:hi], odloc[lo:hi]
        dnw, dn1 = onw[lo:hi], on1[lo:hi]
        srcw, dlocv, nwv, n1v = [], [], [], []
        cell_d = dt * NSUB + ds
        cello = np.searchsorted(cell_d, np.arange(NBLK * NSUB + 1))
        for s in range(NSUB):
            nslot = G_s[s] * SLOTS
            sl = np.zeros(nslot, np.int64)
            dl = np.zeros(nslot, np.float32)
            wv = np.zeros(nslot, np.float32)
            v1 = np.zeros(nslot, np.float32)
            for t in range(NBLK):
                a, b = cello[t * NSUB + s], cello[t * NSUB + s + 1]
                n = b - a
                p0 = base_pos[t, s] * 128
                sl[p0:p0 + n] = dsl[a:b]
                dl[p0:p0 + n] = ddl[a:b]
                wv[p0:p0 + n] = dnw[a:b]
                v1[p0:p0 + n] = dn1[a:b]
            srcw.append(_wrap_idx(sl.astype(np.int16)))
            dlocv.append(dl.reshape(G_s[s], CPG, 128).transpose(0, 2, 1).copy())
            nwv.append(wv.reshape(G_s[s], CPG, 128).transpose(0, 2, 1).copy())
            n1v.append(v1.reshape(G_s[s], CPG, 128).transpose(0, 2, 1).copy())
        # self-loop stream: block t chunk = the shard's own rows t*128..t*128+128,
        # dloc = identity, norms = dinv^2 (0 for dummy rows)
        nself = G5 * CPG * 128
        v_glob = np.arange(d * SH, d * SH + SH, dtype=np.int64)
        swv = np.zeros(nself, np.float32)
        sv1 = np.zeros(nself, np.float32)
        real = v_glob < N
        swv[:SH][real] = (dinv_w * dinv_w)[v_glob[real]]
        sv1[:SH][real] = (dinv_1 * dinv_1)[v_glob[real]]
        sdl = np.tile(np.arange(128, dtype=np.float32), G5 * CPG)
        sdlocv = sdl.reshape(G5, CPG, 128).transpose(0, 2, 1).copy()
        snwv = swv.reshape(G5, CPG, 128).transpose(0, 2, 1).copy()
        sn1v = sv1.reshape(G5, CPG, 128).transpose(0, 2, 1).copy()
        per_core.append((srcw, dlocv, nwv, n1v, sdlocv, snwv, sn1v))

    return K_ts, G_s, base_pos, per_core


def _build(K_ts, G_s, base_pos):
    f32 = mybir.dt.float32
    G5 = int(math.ceil(NBLK / CPG))
    nc = bacc.Bacc(None, target_bir_lowering=False, num_swdge_queues=4, num_devices=NCORES)

    xs_d = nc.dram_tensor("xs", [SH, FIN], f32, kind="ExternalInput")
    srcw_d = [nc.dram_tensor(f"srcw{s}", [G_s[s], 128, SLOTS // 16], mybir.dt.int16, kind="ExternalInput") for s in range(NSUB)]
    dloc_d = [nc.dram_tensor(f"dloc{s}", [G_s[s], 128, CPG], f32, kind="ExternalInput") for s in range(NSUB)]
    nw_d = [nc.dram_tensor(f"nw{s}", [G_s[s], 128, CPG], f32, kind="ExternalInput") for s in range(NSUB)]
    n1_d = [nc.dram_tensor(f"n1{s}", [G_s[s], 128, CPG], f32, kind="ExternalInput") for s in range(NSUB)]
    sdloc_d = nc.dram_tensor("sdloc", [G5, 128, CPG], f32, kind="ExternalInput")
    snw_d = nc.dram_tensor("snw", [G5, 128, CPG], f32, kind="ExternalInput")
    sn1_d = nc.dram_tensor("sn1", [G5, 128, CPG], f32, kind="ExternalInput")
    w1_d = nc.dram_tensor("W1", [FIN, HID], f32, kind="ExternalInput")
    w2_d = nc.dram_tensor("W2", [HID, HID], f32, kind="ExternalInput")
    wm_d = nc.dram_tensor("Wmu", [HID, OUT], f32, kind="ExternalInput")
    wl_d = nc.dram_tensor("Wls", [HID, OUT], f32, kind="ExternalInput")
    b1_d = nc.dram_tensor("b1", [HID, 1], f32, kind="ExternalInput")
    b2_d = nc.dram_tensor("b2", [HID, 1], f32, kind="ExternalInput")
    bm_d = nc.dram_tensor("bmu", [OUT, 1], f32, kind="ExternalInput")
    bl_d = nc.dram_tensor("bls", [OUT, 1], f32, kind="ExternalInput")
    iota_d = nc.dram_tensor("iota8", [128, CPG * 128], f32, kind="ExternalInput")
    ident_d = nc.dram_tensor("ident", [128, 128], f32, kind="ExternalInput")
    out_d = nc.dram_tensor("out", [SH, 2 * OUT], f32, kind="ExternalOutput")

    ag_in = [nc.dram_tensor(f"ag_in{i}", [SH, HID], f32) for i in range(3)]
    tables = [nc.dram_tensor(f"table{i}", [NPAD, HID], f32, addr_space="Shared") for i in range(3)]

    with tile.TileContext(nc) as tc:
        with (
            tc.tile_pool(name="const", bufs=1) as kpool,
            tc.tile_pool(name="idx", bufs=8) as ipool,
            tc.tile_pool(name="meta", bufs=8) as mpool,
            tc.tile_pool(name="g", bufs=2) as gpool,
            tc.tile_pool(name="b", bufs=2) as bpool,
            tc.tile_pool(name="agg", bufs=1) as apool,
            tc.tile_pool(name="selfd", bufs=1) as selfpool,
            tc.tile_pool(name="stage", bufs=1) as spool,
            tc.tile_pool(name="tmp", bufs=4) as tpool,
            tc.tile_pool(name="pagg", bufs=2, space="PSUM") as pagg,
            tc.tile_pool(name="pmm", bufs=2, space="PSUM") as pmm,
            tc.tile_pool(name="ptr", bufs=2, space="PSUM") as ptr,
        ):
            nc.gpsimd.load_library(mlp)

            iota_t = kpool.tile([128, CPG * 128], f32)
            nc.sync.dma_start(iota_t[:], iota_d[:])
            ident_t = kpool.tile([128, 128], f32)
            nc.sync.dma_start(ident_t[:], ident_d[:])
            w1_t = kpool.tile([FIN, HID], f32)
            nc.sync.dma_start(w1_t[:], w1_d[:])
            w2_t = kpool.tile([HID, HID], f32)
            nc.sync.dma_start(w2_t[:], w2_d[:])
            wm_t = kpool.tile([HID, OUT], f32)
            nc.sync.dma_start(wm_t[:], wm_d[:])
            wl_t = kpool.tile([HID, OUT], f32)
            nc.sync.dma_start(wl_t[:], wl_d[:])
            b1_t = kpool.tile([HID, 1], f32)
            nc.sync.dma_start(b1_t[:], b1_d[:])
            b2_t = kpool.tile([HID, 1], f32)
            nc.sync.dma_start(b2_t[:], b2_d[:])
            bm_t = kpool.tile([OUT, 1], f32)
            nc.sync.dma_start(bm_t[:], bm_d[:])
            bl_t = kpool.tile([OUT, 1], f32)
            nc.sync.dma_start(bl_t[:], bl_d[:])

            def build_b(dt_ap, pool_tag):
                bt = bpool.tile([128, CPG, 128], f32, tag=pool_tag)
                nc.vector.tensor_tensor(
                    out=bt[:],
                    in0=iota_t[:].rearrange("p (j v) -> p j v", j=CPG),
                    in1=dt_ap.to_broadcast([128, CPG, 128]),
                    op=mybir.AluOpType.is_equal,
                )
                return bt

            def aggregate(table_i, norm_of_s, snorm_d, agg_t):
                """agg_t[:HID, t*128+v] = sum over edges into (t,v) of coeff*table[src]."""
                F = HID
                cur = {}
                gq = [0]

                # dense self-loop stream: this core's own shard rows
                ds = selfpool.tile([128, NBLK, F], f32, tag="selfd")
                nc.sync.dma_start(
                    ds[:], ag_in[table_i][:].rearrange("(t p) f -> p t f", p=128)
                )
                for g in range(G5):
                    snt = mpool.tile([128, CPG], f32, tag="nm")
                    nc.sync.dma_start(snt[:], snorm_d[g])
                    hi = min(NBLK - g * CPG, CPG)
                    nc.vector.tensor_tensor(
                        out=ds[:, g * CPG:g * CPG + hi, :],
                        in0=ds[:, g * CPG:g * CPG + hi, :],
                        in1=snt[:, :hi].to_broadcast([128, hi, F]),
                        op=mybir.AluOpType.mult,
                    )
                bself = {}  # lazily built, one group alive at a time

                def ensure_self(g):
                    if bself and g in bself:
                        return bself[g]
                    sdt = mpool.tile([128, CPG], f32, tag="dl")
                    nc.sync.dma_start(sdt[:], sdloc_d[g])
                    bself.clear()
                    bself[g] = build_b(sdt[:], "bself")
                    return bself[g]

                def ensure(s, g):
                    if s in cur and cur[s][0] == g:
                        return cur[s][1]
                    it = ipool.tile([128, SLOTS // 16], mybir.dt.int16, tag="idx")
                    nc.sync.dma_start(it[:], srcw_d[s][g])
                    gt = gpool.tile([128, CPG, F], f32, tag=f"g{s}")
                    nc.gpsimd.dma_gather(
                        gt[:], tables[table_i][s * SUB:(s + 1) * SUB, :], it[:],
                        SLOTS, SLOTS, F, queue_num=gq[0] % 4,
                    )
                    gq[0] += 1
                    nt = mpool.tile([128, CPG], f32, tag="nm")
                    nc.sync.dma_start(nt[:], norm_of_s(s)[g])
                    dt_ = mpool.tile([128, CPG], f32, tag="dl")
                    nc.sync.dma_start(dt_[:], dloc_d[s][g])
                    bt = build_b(dt_[:], f"b{s}")
                    nc.vector.tensor_tensor(
                        out=gt[:],
                        in0=gt[:],
                        in1=nt[:].to_broadcast([128, CPG, F]),
                        op=mybir.AluOpType.mult,
                    )
                    cur[s] = (g, (gt, bt))
                    return gt, bt

                for t in range(NBLK):
                    ps = pagg.tile([128, 128], f32, tag="pagg")
                    nchunks = int(K_ts[t].sum()) + 1
                    # self-loop chunk first
                    g5, j5 = divmod(t, CPG)
                    bs = ensure_self(g5)
                    nc.tensor.matmul(
                        ps[:F, :], lhsT=ds[:, t, :], rhs=bs[:, j5, :],
                        start=True, stop=False,
                    )
                    ci = 1
                    for s in range(NSUB):
                        for k in range(int(K_ts[t, s])):
                            pos = int(base_pos[t, s]) + k
                            g, j = divmod(pos, CPG)
                            gt, bt = ensure(s, g)
                            nc.tensor.matmul(
                                ps[:F, :],
                                lhsT=gt[:, j, :],
                                rhs=bt[:, j, :],
                                start=False,
                                stop=(ci == nchunks - 1),
                            )
                            ci += 1
                    nc.vector.tensor_copy(agg_t[:F, t * 128:(t + 1) * 128], ps[:F, :])

            def project_transpose_store(agg_t, w_t, b_t, func, dram_rows, half=None):
                """dram[t*128+p, :] = func(W.T @ agg[:, t*128+p] + b), per block.

                w_t=None: identity projection (agg already in h-space).
                half: (ht, lo) to place result into rows [lo:lo+HID) of ht
                      and defer transpose/store (used by the mu/ls pair).
                """
                st = None if half else spool.tile([128, NBLK * HID], f32, tag="st")
                for t in range(NBLK):
                    if w_t is not None:
                        ph = pmm.tile([w_t.shape[1], 128], f32, tag="pmm")
                        nc.tensor.matmul(
                            ph[:], lhsT=w_t[:], rhs=agg_t[:w_t.shape[0], t * 128:(t + 1) * 128],
                            start=True, stop=True,
                        )
                        src_ap = ph[:]
                    else:
                        src_ap = agg_t[:HID, t * 128:(t + 1) * 128]
                    ht = tpool.tile([HID, 128], f32, tag="ht")
                    if func is None:
                        nc.vector.tensor_scalar_add(ht[:], src_ap, b_t[:])
                    else:
                        nc.scalar.activation(ht[:], src_ap, func, bias=b_t[:])
                    pt = ptr.tile([128, HID], f32, tag="ptr")
                    nc.tensor.transpose(pt[:], ht[:], ident_t[:HID, :HID])
                    nc.vector.tensor_copy(st[:, t * HID:(t + 1) * HID], pt[:])
                nc.sync.dma_start(
                    dram_rows.rearrange("(t p) f -> p t f", p=128),
                    st[:].rearrange("p (t f) -> p t f", f=HID),
                )

            def allgather(i):
                nc.gpsimd.collective_compute(
                    "AllGather", mybir.AluOpType.bypass,
                    replica_groups=[list(range(NCORES))],
                    ins=[ag_in[i][:]], outs=[tables[i][:]],
                )

            # ---- pre-projection: table0 = x_shard @ W1 (no bias) ----
            st0 = spool.tile([128, NBLK * HID], f32, tag="st")
            for t in range(NBLK):
                xt = tpool.tile([128, FIN], f32, tag="xt")
                nc.sync.dma_start(xt[:], xs_d[t * 128:(t + 1) * 128, :])
                pxt = ptr.tile([128, 128], f32, tag="pxt")
                nc.tensor.transpose(pxt[:], xt[:], ident_t[:])
                xT = tpool.tile([128, 128], f32, tag="xT")
                nc.vector.tensor_copy(xT[:], pxt[:])
                ph0 = pmm.tile([HID, 128], f32, tag="pmm")
                nc.tensor.matmul(ph0[:], lhsT=w1_t[:], rhs=xT[:], start=True, stop=True)
                pt0 = ptr.tile([128, HID], f32, tag="ptr")
                h0 = tpool.tile([HID, 128], f32, tag="ht")
                nc.vector.tensor_copy(h0[:], ph0[:])
                nc.tensor.transpose(pt0[:], h0[:], ident_t[:HID, :HID])
                nc.vector.tensor_copy(st0[:, t * HID:(t + 1) * HID], pt0[:])
            nc.sync.dma_start(
                ag_in[0][:].rearrange("(t p) f -> p t f", p=128),
                st0[:].rearrange("p (t f) -> p t f", f=HID),
            )
            allgather(0)

            # ---- layer 1: aggregate projected x, then bias+relu ----
            agg1 = apool.tile([128, NBLK * 128], f32, tag="agg")
            aggregate(0, lambda s: nw_d[s], snw_d, agg1)
            project_transpose_store(agg1, None, b1_t, mybir.ActivationFunctionType.Relu, ag_in[1][:])
            allgather(1)

            # ---- layer 2: aggregate h1, then W2 + bias ----
            agg2 = apool.tile([128, NBLK * 128], f32, tag="agg")
            aggregate(1, lambda s: nw_d[s], snw_d, agg2)
            project_transpose_store(agg2, w2_t, b2_t, None, ag_in[2][:])
            allgather(2)

            # ---- layer 3: aggregate h2; mu/ls projections ----
            agg3 = apool.tile([128, NBLK * 128], f32, tag="agg")
            aggregate(2, lambda s: n1_d[s], sn1_d, agg3)
            st3 = spool.tile([128, NBLK * 2 * OUT], f32, tag="st")
            for t in range(NBLK):
                mlt = tpool.tile([2 * OUT, 128], f32, tag="ht")
                pm = pmm.tile([OUT, 128], f32, tag="pmm")
                nc.tensor.matmul(pm[:], lhsT=wm_t[:], rhs=agg3[:HID, t * 128:(t + 1) * 128], start=True, stop=True)
                nc.vector.tensor_scalar_add(mlt[:OUT, :], pm[:], bm_t[:])
                pl = pmm.tile([OUT, 128], f32, tag="pmm")
                nc.tensor.matmul(pl[:], lhsT=wl_t[:], rhs=agg3[:HID, t * 128:(t + 1) * 128], start=True, stop=True)
                nc.vector.tensor_scalar_add(mlt[OUT:2 * OUT, :], pl[:], bl_t[:])
                pt = ptr.tile([128, 2 * OUT], f32, tag="ptr")
                nc.tensor.transpose(pt[:], mlt[:], ident_t[:2 * OUT, :2 * OUT])
                nc.vector.tensor_copy(st3[:, t * 2 * OUT:(t + 1) * 2 * OUT], pt[:])
            nc.sync.dma_start(
                out_d[:].rearrange("(t p) f -> p t f", p=128),
                st3[:].rearrange("p (t f) -> p t f", f=2 * OUT),
            )

    # Tile round-robins Pool-DMA completion sems over 8 DMASW lanes without
    # queue awareness, but each sem is hardware-locked to the first SWDGE
    # queue that increments it. Rewrite each gather's queue to lane % 4 so
    # every lane's sem is only ever incremented from one queue.
    for fn in nc.m.functions:
        for blk in fn.blocks:
            for ins in blk.instructions:
                if isinstance(ins, mybir.InstDMAGatherAnt) and ins.sync_info:
                    for u in ins.sync_info.on_update:
                        name = getattr(u, "ant_name", "") or ""
                        if name.startswith("DMASW"):
                            ins.queue_num = int(name[5:].split("_")[0]) % 4
                            break

    nc.compile()
    return nc


def _run(inputs, trace=False):
    x = np.asarray(inputs["x"], np.float32)
    K_ts, G_s, base_pos, per_core = _prep(
        x, np.asarray(inputs["edge_index"]), np.asarray(inputs["edge_weight"])
    )
    nc = _build(K_ts, G_s, base_pos)

    x_pad = np.zeros((NPAD, FIN), np.float32)
    x_pad[:N] = x
    iota8 = np.tile(np.arange(128, dtype=np.float32)[None, :], (128, CPG)).reshape(128, CPG * 128)
    shared = {
        "W1": np.asarray(inputs["W1"], np.float32),
        "W2": np.asarray(inputs["W2"], np.float32),
        "Wmu": np.asarray(inputs["Wmu"], np.float32),
        "Wls": np.asarray(inputs["Wls"], np.float32),
        "b1": np.asarray(inputs["b1"], np.float32).reshape(HID, 1),
        "b2": np.asarray(inputs["b2"], np.float32).reshape(HID, 1),
        "bmu": np.asarray(inputs["bmu"], np.float32).reshape(OUT, 1),
        "bls": np.asarray(inputs["bls"], np.float32).reshape(OUT, 1),
        "iota8": iota8,
        "ident": np.eye(128, dtype=np.float32),
    }
    in_maps = []
    for d in range(NCORES):
        srcw, dlocv, nwv, n1v, sdlocv, snwv, sn1v = per_core[d]
        m = dict(shared)
        m["xs"] = x_pad[d * SH:(d + 1) * SH]
        m["sdloc"] = sdlocv
        m["snw"] = snwv
        m["sn1"] = sn1v
        for s in range(NSUB):
            m[f"srcw{s}"] = srcw[s]
            m[f"dloc{s}"] = dlocv[s]
            m[f"nw{s}"] = nwv[s]
            m[f"n1{s}"] = n1v[s]
        in_maps.append(m)

    res = run_bass_kernel_spmd(nc, in_maps, core_ids=list(range(NCORES)), trace=trace)
    full = np.concatenate([res.results[d]["out"] for d in range(NCORES)], axis=0)
    mu = full[:N, :OUT].copy()
    logstd = full[:N, OUT:].copy()
    return (mu, logstd), res


def kernel(**inputs):
    (mu, logstd), _ = _run(inputs, trace=False)
    return mu, logstd
